# revision 6
# baseline (speedup 1.0000x reference)
"""Trainium2 Bass kernel for nn_MultiHeadAttention (B=4, L=S=2048, D=1024, H=16, causal).

Sharding: 8 cores = 4 batches x 2 head-groups (8 heads each).
Per core: project its batch's q/k/v against its group's weight slices,
causal attention for 8 heads, output-projection against Wo column slice.
Host sums the 2 partial outputs per batch (tensor-parallel reduce).

v2 layout notes:
- All input transposes happen on the host (numpy), so every device DMA is
  linear. X arrives as xT [D, T] per tensor; weights pre-transposed.
- Softmax sums are folded into the PV matmul: each head's V tile carries a
  65th column of ones, so ctx PSUM row 64 accumulates sum(P) for free.
- Scores for diagonal blocks are tightened to the causal width; the mask
  multiply zeroes the stale/garbage columns (masks have 0 there).
- Projections, attention and Wo are interleaved chunk-by-chunk so the PE
  instruction stream never drains.

All matmuls bf16 with fp32 PSUM accumulation.
"""

import sys

if "/opt/trn_rl_repo" not in sys.path:
    sys.path.insert(0, "/opt/trn_rl_repo")

import numpy as np
import ml_dtypes

BF16 = ml_dtypes.bfloat16

# Problem constants (hardcoded per harness contract)
B, L, D, H = 4, 2048, 1024, 16
HD = D // H              # 64
NCORES = 8
GROUPS = 2               # head-groups (tensor parallel)
HG = H // GROUPS         # 8 heads per group
DG = HG * HD             # 512 out-dim per group

T = L                    # tokens per core
DM = D                   # model dim
NDC = DM // 128          # 8 contraction chunks
NP = HG // 2             # 4 head pairs
NCH = T // 512           # 4 token chunks
LCH = 512
TB = 128
NT = T // TB             # 16 token tiles
SCALE = 1.0 / np.sqrt(HD)

FULL_CFG = dict(T=T, DM=DM, DG=DG)


def emit_mha(tc, aps):
    import concourse.bass as bass
    from concourse import mybir

    nc = tc.nc
    f32 = mybir.dt.float32
    bf16 = mybir.dt.bfloat16
    Exp = mybir.ActivationFunctionType.Exp

    import contextlib

    ctx = contextlib.ExitStack()
    with ctx:
        wts = ctx.enter_context(tc.tile_pool(name="wts", bufs=1))
        xpool = ctx.enter_context(tc.tile_pool(name="xp", bufs=2))
        vt_pool = ctx.enter_context(tc.tile_pool(name="vt", bufs=1))
        qt_pool = ctx.enter_context(tc.tile_pool(name="qt", bufs=1))
        kt_pool = ctx.enter_context(tc.tile_pool(name="kt", bufs=1))
        pt_pool = ctx.enter_context(tc.tile_pool(name="ptp", bufs=4))
        cpc_pool = ctx.enter_context(tc.tile_pool(name="cpc", bufs=1))
        ctxn_pool = ctx.enter_context(tc.tile_pool(name="ctxn", bufs=2))
        small = ctx.enter_context(tc.tile_pool(name="small", bufs=1))
        osb_pool = ctx.enter_context(tc.tile_pool(name="osb", bufs=2))
        # PSUM: scores 2x2 banks + ctxA 1 + ctxB 1 + proj 2 = 8 banks
        st_ps = ctx.enter_context(tc.tile_pool(name="st_ps", bufs=2, space="PSUM"))
        ctx_ps = ctx.enter_context(tc.tile_pool(name="ctx_ps", bufs=1, space="PSUM"))
        proj_ps = ctx.enter_context(tc.tile_pool(name="proj_ps", bufs=2, space="PSUM"))

        # ---- weight/mask DMAs on the Activation HWDGE queue ----
        wvT, wqT, wkT = [], [], []
        for nm, lst in (("wv", wvT), ("wq", wqT), ("wk", wkT)):
            for c in range(NDC):
                t = wts.tile([128, DG], bf16, tag=f"{nm}{c}", name=f"{nm}{c}")
                nc.scalar.dma_start(out=t[:], in_=aps[nm + "T"][c * 128:(c + 1) * 128, :])
                lst.append(t)
        woTA, woTB = [], []
        for p in range(NP):
            ta = wts.tile([64, DM], bf16, tag=f"woA{p}", name=f"woA{p}")
            nc.scalar.dma_start(out=ta[:], in_=aps["woT"][p * 128:p * 128 + 64, :])
            woTA.append(ta)
            tb = wts.tile([64, DM], bf16, tag=f"woB{p}", name=f"woB{p}")
            nc.scalar.dma_start(out=tb[:], in_=aps["woT"][p * 128 + 64:(p + 1) * 128, :])
            woTB.append(tb)
        masks = []
        for r in range(4):
            mt = wts.tile([TB, 2 * LCH], bf16, tag=f"mask{r}", name=f"mask{r}")
            nc.scalar.dma_start(out=mt[:], in_=aps["maskt"][r])
            masks.append(mt)

        # pre-zero the pt buffers (stale cols are mask-multiplied; NaN*0=NaN)
        pt_boot = []
        for _ in range(4):
            pt = pt_pool.tile([128, 2 * LCH], bf16, tag="pt")
            nc.vector.memset(pt[:], 0.0)
            pt_boot.append(pt)

        vt = [None] * NT            # [128, HG, HD+1] V tiles (ones in col HD)
        QT = [[None] * NCH for _ in range(NP)]
        KT = [[None] * NCH for _ in range(NP)]

        pending = [None]            # deferred normalize closure

        def attn_pair(p, i):
            """Causal attention for head-pair p over l-chunk i."""
            jmax = 4 * i + 3
            QTi = QT[p][i]
            ctxA = ctx_ps.tile([65, LCH], f32, tag="ctxA")
            ctxB = ctx_ps.tile([65, LCH], f32, tag="ctxB")
            pts = {}

            def sc_act(j):
                r = j - 4 * i
                off = 128 * r if r > 0 else 0
                sp = st_ps.tile([128, 2 * LCH], f32, tag="st")
                jn, jo = j // 4, (j % 4) * 128
                KTj = KT[p][jn]
                nc.tensor.matmul(sp[:, off:LCH], lhsT=KTj[0:64, jo:jo + 128],
                                 rhs=QTi[0:64, off:LCH], start=True, stop=True)
                nc.tensor.matmul(sp[:, LCH + off:2 * LCH], lhsT=KTj[64:128, jo:jo + 128],
                                 rhs=QTi[64:128, off:LCH], start=True, stop=True)
                pt = pt_pool.tile([128, 2 * LCH], bf16, tag="pt")
                if off == 0:
                    nc.scalar.activation(pt[:], sp[:], Exp, scale=float(SCALE))
                else:
                    nc.scalar.activation(pt[:, off:LCH], sp[:, off:LCH], Exp,
                                         scale=float(SCALE))
                    nc.scalar.activation(pt[:, LCH + off:2 * LCH],
                                         sp[:, LCH + off:2 * LCH], Exp,
                                         scale=float(SCALE))
                if r >= 0:
                    nc.vector.tensor_mul(pt[:], pt[:], masks[r][:])
                pts[j] = (pt, off)

            def pv(j):
                pt, off = pts.pop(j)
                st_f = (j == 0)
                en = (j == jmax)
                nc.tensor.matmul(ctxA[:, off:LCH], lhsT=vt[j][:, 2 * p, :],
                                 rhs=pt[:, off:LCH], start=st_f, stop=en,
                                 skip_group_check=True)
                nc.tensor.matmul(ctxB[:, off:LCH], lhsT=vt[j][:, 2 * p + 1, :],
                                 rhs=pt[:, LCH + off:2 * LCH], start=st_f, stop=en,
                                 skip_group_check=True)

            sc_act(0)
            for j in range(jmax + 1):
                if j + 1 <= jmax:
                    sc_act(j + 1)
                pv(j)
                if j == 1 and pending[0] is not None:
                    pending[0]()
                    pending[0] = None
            # drain ctx+sums to SBUF on the Pool engine
            cA = cpc_pool.tile([65, LCH], f32, tag=f"cpcA{p}", name=f"cpcA{p}")
            cB = cpc_pool.tile([65, LCH], f32, tag=f"cpcB{p}", name=f"cpcB{p}")
            nc.vector.tensor_copy(cA[:], ctxA[:])
            nc.vector.tensor_copy(cB[:], ctxB[:])
            return cA, cB

        def make_normalize(p, cA, cB, out_slot):
            def run():
                rec = small.tile([1, 2 * LCH], f32, tag="rec")
                nc.gpsimd.dma_start(out=rec[0:1, 0:LCH], in_=cA[64:65, :])
                nc.gpsimd.dma_start(out=rec[0:1, LCH:2 * LCH], in_=cB[64:65, :])
                recr = small.tile([1, 2 * LCH], f32, tag="recr")
                nc.vector.reciprocal(recr[:], rec[:])
                rb = small.tile([128, 2 * LCH], f32, tag="rb")
                nc.gpsimd.partition_broadcast(rb[:], recr[0:1, :])
                tA = ctxn_pool.tile([64, LCH], bf16, tag=f"cnA{p}", name=f"cnA{p}")
                tB = ctxn_pool.tile([64, LCH], bf16, tag=f"cnB{p}", name=f"cnB{p}")
                nc.gpsimd.tensor_mul(tA[:], cA[0:64, :], rb[0:64, 0:LCH])
                nc.gpsimd.tensor_mul(tB[:], cB[0:64, :], rb[0:64, LCH:2 * LCH])
                out_slot[p] = (tA, tB)
            return run

        def emit_wo(m, ctxn):
            """Output projection for l-chunk m using normalized ctx tiles."""
            for ltl in range(4):
                lt = 4 * m + ltl
                osb = osb_pool.tile([128, DM], f32, tag="osb")
                for oc in range(2):
                    ps = proj_ps.tile([128, LCH], f32, tag="proj")
                    k = 0
                    for p in range(NP):
                        tA, tB = ctxn[p]
                        nc.tensor.matmul(ps[:], lhsT=tA[:, ltl * 128:(ltl + 1) * 128],
                                         rhs=woTA[p][:, oc * LCH:(oc + 1) * LCH],
                                         start=(k == 0), stop=False)
                        k += 1
                        nc.tensor.matmul(ps[:], lhsT=tB[:, ltl * 128:(ltl + 1) * 128],
                                         rhs=woTB[p][:, oc * LCH:(oc + 1) * LCH],
                                         start=False, stop=(k == 2 * NP - 1))
                        k += 1
                    nc.vector.tensor_copy(osb[:, oc * LCH:(oc + 1) * LCH], ps[:])
                nc.sync.dma_start(out=aps["y"][lt * TB:(lt + 1) * TB, :], in_=osb[:])

        ctxn_prev = None
        for n in range(NCH):
            # ---- x DMAs for chunk n (sync queue), then projections ----
            xv_n = []
            for c in range(NDC):
                t = xpool.tile([128, LCH], bf16, tag=f"xv{c}", name=f"xv{c}_{n}")
                nc.sync.dma_start(out=t[:], in_=aps["xvT"][c * 128:(c + 1) * 128,
                                                           n * LCH:(n + 1) * LCH])
                xv_n.append(t)
            for stl in range(4):
                st = 4 * n + stl
                ps = proj_ps.tile([128, DG], f32, tag="proj")
                for c in range(NDC):
                    nc.tensor.matmul(ps[:], lhsT=xv_n[c][:, stl * 128:(stl + 1) * 128],
                                     rhs=wvT[c][:], start=(c == 0), stop=(c == NDC - 1))
                v = vt_pool.tile([128, HG, HD + 1], bf16, tag=f"V{st}", name=f"V{st}")
                nc.vector.tensor_copy(v[:, :, 0:HD],
                                      ps[:].rearrange("a (b c) -> a b c", b=HG))
                nc.vector.memset(v[:, :, HD:HD + 1], 1.0)
                vt[st] = v

            xq_n = []
            for c in range(NDC):
                t = xpool.tile([128, LCH], bf16, tag=f"xq{c}", name=f"xq{c}_{n}")
                nc.sync.dma_start(out=t[:], in_=aps["xqT"][c * 128:(c + 1) * 128,
                                                           n * LCH:(n + 1) * LCH])
                xq_n.append(t)
            for m in range(NP):
                ps = proj_ps.tile([128, LCH], f32, tag="proj")
                for c in range(NDC):
                    nc.tensor.matmul(ps[:], lhsT=wqT[c][:, m * 128:(m + 1) * 128],
                                     rhs=xq_n[c][:], start=(c == 0), stop=(c == NDC - 1))
                qt = qt_pool.tile([128, LCH], bf16, tag=f"QT{m}_{n}", name=f"QT{m}_{n}")
                nc.vector.tensor_copy(qt[:], ps[:])
                QT[m][n] = qt

            xk_n = []
            for c in range(NDC):
                t = xpool.tile([128, LCH], bf16, tag=f"xk{c}", name=f"xk{c}_{n}")
                nc.sync.dma_start(out=t[:], in_=aps["xkT"][c * 128:(c + 1) * 128,
                                                           n * LCH:(n + 1) * LCH])
                xk_n.append(t)
            for p in range(NP):
                ps = proj_ps.tile([128, LCH], f32, tag="proj")
                for c in range(NDC):
                    nc.tensor.matmul(ps[:], lhsT=wkT[c][:, p * 128:(p + 1) * 128],
                                     rhs=xk_n[c][:], start=(c == 0), stop=(c == NDC - 1))
                kt = kt_pool.tile([128, LCH], bf16, tag=f"KT{p}_{n}", name=f"KT{p}_{n}")
                nc.vector.tensor_copy(kt[:], ps[:])
                KT[p][n] = kt

            # ---- attention for l-chunk n, Wo for chunk n-1 after pair 0 ----
            ctxn_cur = [None] * NP
            for p in range(NP):
                cA, cB = attn_pair(p, n)
                pending[0] = make_normalize(p, cA, cB, ctxn_cur)
                if p == 0 and n > 0:
                    emit_wo(n - 1, ctxn_prev)
            ctxn_prev = ctxn_cur

        # tail: last pair's normalize + last chunk's Wo
        if pending[0] is not None:
            pending[0]()
            pending[0] = None
        emit_wo(NCH - 1, ctxn_prev)


def make_mask_tiles(cfg):
    T_, LCH_, TB_ = cfg["T"], min(512, cfg["T"]), 128
    nMask = LCH_ // TB_
    f = np.arange(2 * LCH_) % LCH_
    p = np.arange(TB_)
    tiles = []
    for r in range(nMask):
        m = (f[None, :] >= (TB_ * r + p)[:, None]).astype(np.float32)
        tiles.append(m)
    return np.stack(tiles).astype(BF16)


def build_nc(cfg):
    """Build and compile the per-core Bass program. Returns nc."""
    import concourse.bacc as bacc
    import concourse.tile as tile
    from concourse import mybir

    T_, DM_, DG_ = cfg["T"], cfg["DM"], cfg["DG"]

    nc = bacc.Bacc("TRN2", target_bir_lowering=False, debug=False)
    f32 = mybir.dt.float32
    bf16 = mybir.dt.bfloat16
    aps = {}
    for nm, shape, dt in [
        ("xqT", [DM_, T_], bf16), ("xkT", [DM_, T_], bf16), ("xvT", [DM_, T_], bf16),
        ("wqT", [DM_, DG_], bf16), ("wkT", [DM_, DG_], bf16), ("wvT", [DM_, DG_], bf16),
        ("woT", [DG_, DM_], bf16),
        ("maskt", [4, 128, 1024], bf16),
    ]:
        aps[nm] = nc.dram_tensor(nm, shape, dt, kind="ExternalInput").ap()
    aps["y"] = nc.dram_tensor("y", [T_, DM_], f32, kind="ExternalOutput").ap()

    with tile.TileContext(nc) as tc:
        emit_mha(tc, aps)
    nc.compile()
    return nc


_CACHE = {}


def _get_nc():
    if "nc" not in _CACHE:
        _CACHE["nc"] = build_nc(FULL_CFG)
    return _CACHE["nc"]


def shard_inputs(q, k, v, Wq, Wk, Wv, Wo):
    """Build the per-core input maps (8 cores = 4 batches x 2 groups)."""
    maskt = make_mask_tiles(FULL_CFG)
    xT = {}
    for b in range(B):
        xT[b] = (np.ascontiguousarray(q[b].T).astype(BF16),
                 np.ascontiguousarray(k[b].T).astype(BF16),
                 np.ascontiguousarray(v[b].T).astype(BF16))
    wT = {}
    for g in range(GROUPS):
        rows = slice(g * DG, (g + 1) * DG)
        wT[g] = (np.ascontiguousarray(Wq[rows].T).astype(BF16),
                 np.ascontiguousarray(Wk[rows].T).astype(BF16),
                 np.ascontiguousarray(Wv[rows].T).astype(BF16),
                 np.ascontiguousarray(Wo[:, rows].T).astype(BF16))
    in_maps = []
    for core in range(NCORES):
        b, g = divmod(core, GROUPS)
        xqT, xkT, xvT = xT[b]
        wqT, wkT, wvT, woT = wT[g]
        in_maps.append({
            "xqT": xqT, "xkT": xkT, "xvT": xvT,
            "wqT": wqT, "wkT": wkT, "wvT": wvT, "woT": woT,
            "maskt": maskt,
        })
    return in_maps


def kernel(q, k, v, mask, Wq, Wk, Wv, Wo):
    from concourse import bass_utils

    q = np.asarray(q, dtype=np.float32)
    k = np.asarray(k, dtype=np.float32)
    v = np.asarray(v, dtype=np.float32)
    Wq = np.asarray(Wq, dtype=np.float32)
    Wk = np.asarray(Wk, dtype=np.float32)
    Wv = np.asarray(Wv, dtype=np.float32)
    Wo = np.asarray(Wo, dtype=np.float32)

    nc = _get_nc()
    in_maps = shard_inputs(q, k, v, Wq, Wk, Wv, Wo)
    res = bass_utils.run_bass_kernel_spmd(nc, in_maps, core_ids=list(range(NCORES)))
    out = np.zeros((B, L, D), dtype=np.float32)
    for core in range(NCORES):
        b = core // GROUPS
        out[b] += res.results[core]["y"]
    return out


# revision 8
# speedup vs baseline: 1.1999x; 1.1999x over previous
"""Trainium2 Bass kernel for nn_MultiHeadAttention (B=4, L=S=2048, D=1024, H=16, causal).

Sharding: 8 cores = 4 batches x 2 head-groups (8 heads each).
Per core: project its batch's q/k/v against its group's weight slices,
causal attention for 8 heads, output-projection against Wo column slice.
Host sums the 2 partial outputs per batch (tensor-parallel reduce).

v2 layout notes:
- All input transposes happen on the host (numpy), so every device DMA is
  linear. X arrives as xT [D, T] per tensor; weights pre-transposed.
- Softmax sums are folded into the PV matmul: each head's V tile carries a
  65th column of ones, so ctx PSUM row 64 accumulates sum(P) for free.
- Scores for diagonal blocks are tightened to the causal width; the mask
  multiply zeroes the stale/garbage columns (masks have 0 there).
- Projections, attention and Wo are interleaved chunk-by-chunk so the PE
  instruction stream never drains.

All matmuls bf16 with fp32 PSUM accumulation.
"""

import sys

if "/opt/trn_rl_repo" not in sys.path:
    sys.path.insert(0, "/opt/trn_rl_repo")

import numpy as np
import ml_dtypes

BF16 = ml_dtypes.bfloat16

# Problem constants (hardcoded per harness contract)
B, L, D, H = 4, 2048, 1024, 16
HD = D // H              # 64
NCORES = 8
GROUPS = 2               # head-groups (tensor parallel)
HG = H // GROUPS         # 8 heads per group
DG = HG * HD             # 512 out-dim per group

T = L                    # tokens per core
DM = D                   # model dim
NDC = DM // 128          # 8 contraction chunks
NP = HG // 2             # 4 head pairs
NCH = T // 512           # 4 token chunks
LCH = 512
TB = 128
NT = T // TB             # 16 token tiles
SCALE = 1.0 / np.sqrt(HD)

FULL_CFG = dict(T=T, DM=DM, DG=DG)


def emit_mha(tc, aps):
    import concourse.bass as bass
    from concourse import mybir

    nc = tc.nc
    f32 = mybir.dt.float32
    bf16 = mybir.dt.bfloat16
    Exp = mybir.ActivationFunctionType.Exp

    import contextlib

    ctx = contextlib.ExitStack()
    with ctx:
        wts = ctx.enter_context(tc.tile_pool(name="wts", bufs=1))
        xpool = ctx.enter_context(tc.tile_pool(name="xp", bufs=2))
        vt_pool = ctx.enter_context(tc.tile_pool(name="vt", bufs=1))
        qt_pool = ctx.enter_context(tc.tile_pool(name="qt", bufs=1))
        kt_pool = ctx.enter_context(tc.tile_pool(name="kt", bufs=1))
        pt_pool = ctx.enter_context(tc.tile_pool(name="ptp", bufs=4))
        cpc_pool = ctx.enter_context(tc.tile_pool(name="cpc", bufs=1))
        ctxn_pool = ctx.enter_context(tc.tile_pool(name="ctxn", bufs=2))
        small = ctx.enter_context(tc.tile_pool(name="small", bufs=1))
        osb_pool = ctx.enter_context(tc.tile_pool(name="osb", bufs=2))
        # PSUM: scores 2x2 banks + ctxA 1 + ctxB 1 + proj 2 = 8 banks
        st_ps = ctx.enter_context(tc.tile_pool(name="st_ps", bufs=2, space="PSUM"))
        ctx_ps = ctx.enter_context(tc.tile_pool(name="ctx_ps", bufs=1, space="PSUM"))
        proj_ps = ctx.enter_context(tc.tile_pool(name="proj_ps", bufs=2, space="PSUM"))

        # ---- weight/mask DMAs on the Activation HWDGE queue ----
        wvT, wqT, wkT = [], [], []
        for nm, lst in (("wv", wvT), ("wq", wqT), ("wk", wkT)):
            for c in range(NDC):
                t = wts.tile([128, DG], bf16, tag=f"{nm}{c}", name=f"{nm}{c}")
                nc.scalar.dma_start(out=t[:], in_=aps[nm + "T"][c * 128:(c + 1) * 128, :])
                lst.append(t)
        woTA, woTB = [], []
        for p in range(NP):
            ta = wts.tile([64, DM], bf16, tag=f"woA{p}", name=f"woA{p}")
            nc.scalar.dma_start(out=ta[:], in_=aps["woT"][p * 128:p * 128 + 64, :])
            woTA.append(ta)
            tb = wts.tile([64, DM], bf16, tag=f"woB{p}", name=f"woB{p}")
            nc.scalar.dma_start(out=tb[:], in_=aps["woT"][p * 128 + 64:(p + 1) * 128, :])
            woTB.append(tb)
        masks = []
        for r in range(4):
            mt = wts.tile([TB, 2 * LCH], bf16, tag=f"mask{r}", name=f"mask{r}")
            nc.scalar.dma_start(out=mt[:], in_=aps["maskt"][r])
            masks.append(mt)

        # pre-zero the pt buffers (stale cols are mask-multiplied; NaN*0=NaN)
        pt_boot = []
        for _ in range(4):
            pt = pt_pool.tile([128, 2 * LCH], bf16, tag="pt")
            nc.vector.memset(pt[:], 0.0)
            pt_boot.append(pt)

        vt = [None] * NT            # [128, HG, HD+1] V tiles (ones in col HD)
        QT = [[None] * NCH for _ in range(NP)]
        KT = [[None] * NCH for _ in range(NP)]

        pending = [None]            # deferred normalize closure

        def attn_pair(p, i):
            """Causal attention for head-pair p over l-chunk i."""
            jmax = 4 * i + 3
            QTi = QT[p][i]
            ctxA = ctx_ps.tile([65, LCH], f32, tag="ctxA")
            ctxB = ctx_ps.tile([65, LCH], f32, tag="ctxB")
            pts = {}

            def sc_act(j):
                r = j - 4 * i
                off = 128 * r if r > 0 else 0
                sp = st_ps.tile([128, 2 * LCH], f32, tag="st")
                jn, jo = j // 4, (j % 4) * 128
                KTj = KT[p][jn]
                nc.tensor.matmul(sp[:, off:LCH], lhsT=KTj[0:64, jo:jo + 128],
                                 rhs=QTi[0:64, off:LCH], start=True, stop=True)
                nc.tensor.matmul(sp[:, LCH + off:2 * LCH], lhsT=KTj[64:128, jo:jo + 128],
                                 rhs=QTi[64:128, off:LCH], start=True, stop=True)
                pt = pt_pool.tile([128, 2 * LCH], bf16, tag="pt")
                if off == 0:
                    nc.scalar.activation(pt[:], sp[:], Exp, scale=float(SCALE))
                else:
                    nc.scalar.activation(pt[:, off:LCH], sp[:, off:LCH], Exp,
                                         scale=float(SCALE))
                    nc.scalar.activation(pt[:, LCH + off:2 * LCH],
                                         sp[:, LCH + off:2 * LCH], Exp,
                                         scale=float(SCALE))
                if r >= 0:
                    # mask cols [0:128(r+1)) per head: zeroes the causal
                    # triangle plus any stale columns; the rest is all-ones
                    w = 128 * (r + 1)
                    nc.vector.tensor_mul(pt[:, 0:w], pt[:, 0:w], masks[r][:, 0:w])
                    nc.vector.tensor_mul(pt[:, LCH:LCH + w], pt[:, LCH:LCH + w],
                                         masks[r][:, LCH:LCH + w])
                pts[j] = (pt, off)

            def pv(j):
                pt, off = pts.pop(j)
                st_f = (j == 0)
                en = (j == jmax)
                nc.tensor.matmul(ctxA[:, off:LCH], lhsT=vt[j][:, 2 * p, :],
                                 rhs=pt[:, off:LCH], start=st_f, stop=en,
                                 skip_group_check=True)
                nc.tensor.matmul(ctxB[:, off:LCH], lhsT=vt[j][:, 2 * p + 1, :],
                                 rhs=pt[:, LCH + off:2 * LCH], start=st_f, stop=en,
                                 skip_group_check=True)

            sc_act(0)
            for j in range(jmax + 1):
                if j + 1 <= jmax:
                    sc_act(j + 1)
                pv(j)
                if j == 1 and pending[0] is not None:
                    pending[0]()
                    pending[0] = None
            # drain ctx+sums to SBUF on the Pool engine
            cA = cpc_pool.tile([65, LCH], f32, tag=f"cpcA{p}", name=f"cpcA{p}")
            cB = cpc_pool.tile([65, LCH], f32, tag=f"cpcB{p}", name=f"cpcB{p}")
            nc.vector.tensor_copy(cA[:], ctxA[:])
            nc.vector.tensor_copy(cB[:], ctxB[:])
            return cA, cB

        def make_normalize(p, cA, cB, out_slot):
            def run():
                rec = small.tile([1, 2 * LCH], f32, tag="rec")
                nc.gpsimd.dma_start(out=rec[0:1, 0:LCH], in_=cA[64:65, :])
                nc.gpsimd.dma_start(out=rec[0:1, LCH:2 * LCH], in_=cB[64:65, :])
                recr = small.tile([1, 2 * LCH], f32, tag="recr")
                nc.vector.reciprocal_approx_fast(out=recr[:], in_=rec[:])
                rb = small.tile([128, 2 * LCH], f32, tag="rb")
                nc.gpsimd.partition_broadcast(rb[:], recr[0:1, :])
                tA = ctxn_pool.tile([64, LCH], bf16, tag=f"cnA{p}", name=f"cnA{p}")
                tB = ctxn_pool.tile([64, LCH], bf16, tag=f"cnB{p}", name=f"cnB{p}")
                nc.gpsimd.tensor_mul(tA[:], cA[0:64, :], rb[0:64, 0:LCH])
                nc.gpsimd.tensor_mul(tB[:], cB[0:64, :], rb[0:64, LCH:2 * LCH])
                out_slot[p] = (tA, tB)
            return run

        def emit_wo(m, ctxn):
            """Output projection for l-chunk m using normalized ctx tiles."""
            for ltl in range(4):
                lt = 4 * m + ltl
                osb = osb_pool.tile([128, DM], f32, tag="osb")
                for oc in range(2):
                    ps = proj_ps.tile([128, LCH], f32, tag="proj")
                    k = 0
                    for p in range(NP):
                        tA, tB = ctxn[p]
                        nc.tensor.matmul(ps[:], lhsT=tA[:, ltl * 128:(ltl + 1) * 128],
                                         rhs=woTA[p][:, oc * LCH:(oc + 1) * LCH],
                                         start=(k == 0), stop=False)
                        k += 1
                        nc.tensor.matmul(ps[:], lhsT=tB[:, ltl * 128:(ltl + 1) * 128],
                                         rhs=woTB[p][:, oc * LCH:(oc + 1) * LCH],
                                         start=False, stop=(k == 2 * NP - 1))
                        k += 1
                    nc.vector.tensor_copy(osb[:, oc * LCH:(oc + 1) * LCH], ps[:])
                nc.sync.dma_start(out=aps["y"][lt * TB:(lt + 1) * TB, :], in_=osb[:])

        ctxn_prev = None
        for n in range(NCH):
            # ---- x DMAs for chunk n (sync queue), then projections ----
            xv_n = []
            for c in range(NDC):
                t = xpool.tile([128, LCH], bf16, tag=f"xv{c}", name=f"xv{c}_{n}")
                nc.sync.dma_start(out=t[:], in_=aps["xvT"][c * 128:(c + 1) * 128,
                                                           n * LCH:(n + 1) * LCH])
                xv_n.append(t)
            for stl in range(4):
                st = 4 * n + stl
                ps = proj_ps.tile([128, DG], f32, tag="proj")
                for c in range(NDC):
                    nc.tensor.matmul(ps[:], lhsT=xv_n[c][:, stl * 128:(stl + 1) * 128],
                                     rhs=wvT[c][:], start=(c == 0), stop=(c == NDC - 1))
                v = vt_pool.tile([128, HG, HD + 1], bf16, tag=f"V{st}", name=f"V{st}")
                nc.vector.tensor_copy(v[:, :, 0:HD],
                                      ps[:].rearrange("a (b c) -> a b c", b=HG))
                nc.vector.memset(v[:, :, HD:HD + 1], 1.0)
                vt[st] = v

            xq_n = []
            for c in range(NDC):
                t = xpool.tile([128, LCH], bf16, tag=f"xq{c}", name=f"xq{c}_{n}")
                nc.sync.dma_start(out=t[:], in_=aps["xqT"][c * 128:(c + 1) * 128,
                                                           n * LCH:(n + 1) * LCH])
                xq_n.append(t)
            for m in range(NP):
                ps = proj_ps.tile([128, LCH], f32, tag="proj")
                for c in range(NDC):
                    nc.tensor.matmul(ps[:], lhsT=wqT[c][:, m * 128:(m + 1) * 128],
                                     rhs=xq_n[c][:], start=(c == 0), stop=(c == NDC - 1))
                qt = qt_pool.tile([128, LCH], bf16, tag=f"QT{m}_{n}", name=f"QT{m}_{n}")
                nc.vector.tensor_copy(qt[:], ps[:])
                QT[m][n] = qt

            xk_n = []
            for c in range(NDC):
                t = xpool.tile([128, LCH], bf16, tag=f"xk{c}", name=f"xk{c}_{n}")
                nc.sync.dma_start(out=t[:], in_=aps["xkT"][c * 128:(c + 1) * 128,
                                                           n * LCH:(n + 1) * LCH])
                xk_n.append(t)
            for p in range(NP):
                ps = proj_ps.tile([128, LCH], f32, tag="proj")
                for c in range(NDC):
                    nc.tensor.matmul(ps[:], lhsT=wkT[c][:, p * 128:(p + 1) * 128],
                                     rhs=xk_n[c][:], start=(c == 0), stop=(c == NDC - 1))
                kt = kt_pool.tile([128, LCH], bf16, tag=f"KT{p}_{n}", name=f"KT{p}_{n}")
                nc.vector.tensor_copy(kt[:], ps[:])
                KT[p][n] = kt

            # ---- attention for l-chunk n, Wo for chunk n-1 after pair 0 ----
            ctxn_cur = [None] * NP
            for p in range(NP):
                cA, cB = attn_pair(p, n)
                pending[0] = make_normalize(p, cA, cB, ctxn_cur)
                if p == 0 and n > 0:
                    emit_wo(n - 1, ctxn_prev)
            ctxn_prev = ctxn_cur

        # tail: last pair's normalize + last chunk's Wo
        if pending[0] is not None:
            pending[0]()
            pending[0] = None
        emit_wo(NCH - 1, ctxn_prev)


def make_mask_tiles(cfg):
    T_, LCH_, TB_ = cfg["T"], min(512, cfg["T"]), 128
    nMask = LCH_ // TB_
    f = np.arange(2 * LCH_) % LCH_
    p = np.arange(TB_)
    tiles = []
    for r in range(nMask):
        m = (f[None, :] >= (TB_ * r + p)[:, None]).astype(np.float32)
        tiles.append(m)
    return np.stack(tiles).astype(BF16)


def build_nc(cfg):
    """Build and compile the per-core Bass program. Returns nc."""
    import concourse.bacc as bacc
    import concourse.tile as tile
    from concourse import mybir

    T_, DM_, DG_ = cfg["T"], cfg["DM"], cfg["DG"]

    nc = bacc.Bacc("TRN2", target_bir_lowering=False, debug=False)
    f32 = mybir.dt.float32
    bf16 = mybir.dt.bfloat16
    aps = {}
    for nm, shape, dt in [
        ("xqT", [DM_, T_], bf16), ("xkT", [DM_, T_], bf16), ("xvT", [DM_, T_], bf16),
        ("wqT", [DM_, DG_], bf16), ("wkT", [DM_, DG_], bf16), ("wvT", [DM_, DG_], bf16),
        ("woT", [DG_, DM_], bf16),
        ("maskt", [4, 128, 1024], bf16),
    ]:
        aps[nm] = nc.dram_tensor(nm, shape, dt, kind="ExternalInput").ap()
    aps["y"] = nc.dram_tensor("y", [T_, DM_], f32, kind="ExternalOutput").ap()

    with tile.TileContext(nc) as tc:
        emit_mha(tc, aps)
    nc.compile()
    return nc


_CACHE = {}


def _get_nc():
    if "nc" not in _CACHE:
        _CACHE["nc"] = build_nc(FULL_CFG)
    return _CACHE["nc"]


def shard_inputs(q, k, v, Wq, Wk, Wv, Wo):
    """Build the per-core input maps (8 cores = 4 batches x 2 groups)."""
    maskt = make_mask_tiles(FULL_CFG)
    xT = {}
    for b in range(B):
        xT[b] = (np.ascontiguousarray(q[b].T).astype(BF16),
                 np.ascontiguousarray(k[b].T).astype(BF16),
                 np.ascontiguousarray(v[b].T).astype(BF16))
    wT = {}
    for g in range(GROUPS):
        rows = slice(g * DG, (g + 1) * DG)
        wT[g] = (np.ascontiguousarray(Wq[rows].T).astype(BF16),
                 np.ascontiguousarray(Wk[rows].T).astype(BF16),
                 np.ascontiguousarray(Wv[rows].T).astype(BF16),
                 np.ascontiguousarray(Wo[:, rows].T).astype(BF16))
    in_maps = []
    for core in range(NCORES):
        b, g = divmod(core, GROUPS)
        xqT, xkT, xvT = xT[b]
        wqT, wkT, wvT, woT = wT[g]
        in_maps.append({
            "xqT": xqT, "xkT": xkT, "xvT": xvT,
            "wqT": wqT, "wkT": wkT, "wvT": wvT, "woT": woT,
            "maskt": maskt,
        })
    return in_maps


def kernel(q, k, v, mask, Wq, Wk, Wv, Wo):
    from concourse import bass_utils

    q = np.asarray(q, dtype=np.float32)
    k = np.asarray(k, dtype=np.float32)
    v = np.asarray(v, dtype=np.float32)
    Wq = np.asarray(Wq, dtype=np.float32)
    Wk = np.asarray(Wk, dtype=np.float32)
    Wv = np.asarray(Wv, dtype=np.float32)
    Wo = np.asarray(Wo, dtype=np.float32)

    nc = _get_nc()
    in_maps = shard_inputs(q, k, v, Wq, Wk, Wv, Wo)
    res = bass_utils.run_bass_kernel_spmd(nc, in_maps, core_ids=list(range(NCORES)))
    out = np.zeros((B, L, D), dtype=np.float32)
    for core in range(NCORES):
        b = core // GROUPS
        out[b] += res.results[core]["y"]
    return out


# revision 12
# speedup vs baseline: 1.4820x; 1.2351x over previous
"""Trainium2 Bass kernel for nn_MultiHeadAttention (B=4, L=S=2048, D=1024, H=16, causal).

Sharding: 8 cores = 4 batches x 2 head-groups (8 heads each).
Per core: project its batch's q/k/v against its group's weight slices,
causal attention for 8 heads, output-projection against Wo column slice.
Host sums the 2 partial outputs per batch (tensor-parallel reduce).

v2 layout notes:
- All input transposes happen on the host (numpy), so every device DMA is
  linear. X arrives as xT [D, T] per tensor; weights pre-transposed.
- Softmax sums are folded into the PV matmul: each head's V tile carries a
  65th column of ones, so ctx PSUM row 64 accumulates sum(P) for free.
- Scores for diagonal blocks are tightened to the causal width; the mask
  multiply zeroes the stale/garbage columns (masks have 0 there).
- Projections, attention and Wo are interleaved chunk-by-chunk so the PE
  instruction stream never drains.

All matmuls bf16 with fp32 PSUM accumulation.
"""

import sys

if "/opt/trn_rl_repo" not in sys.path:
    sys.path.insert(0, "/opt/trn_rl_repo")

import numpy as np
import ml_dtypes

BF16 = ml_dtypes.bfloat16

# Problem constants (hardcoded per harness contract)
B, L, D, H = 4, 2048, 1024, 16
HD = D // H              # 64
NCORES = 8
GROUPS = 2               # head-groups (tensor parallel)
HG = H // GROUPS         # 8 heads per group
DG = HG * HD             # 512 out-dim per group

T = L                    # tokens per core
DM = D                   # model dim
NDC = DM // 128          # 8 contraction chunks
NP = HG // 2             # 4 head pairs
NCH = T // 512           # 4 token chunks
LCH = 512
TB = 128
NT = T // TB             # 16 token tiles
SCALE = 1.0 / np.sqrt(HD)

FULL_CFG = dict(T=T, DM=DM, DG=DG)


def emit_mha(tc, aps):
    import concourse.bass as bass
    from concourse import mybir

    nc = tc.nc
    f32 = mybir.dt.float32
    bf16 = mybir.dt.bfloat16
    Exp = mybir.ActivationFunctionType.Exp

    import contextlib

    ctx = contextlib.ExitStack()
    with ctx:
        wts = ctx.enter_context(tc.tile_pool(name="wts", bufs=1))
        xpool = ctx.enter_context(tc.tile_pool(name="xp", bufs=2))
        vt_pool = ctx.enter_context(tc.tile_pool(name="vt", bufs=1))
        qt_pool = ctx.enter_context(tc.tile_pool(name="qt", bufs=1))
        kt_pool = ctx.enter_context(tc.tile_pool(name="kt", bufs=1))
        pt_pool = ctx.enter_context(tc.tile_pool(name="ptp", bufs=4))
        cpc_pool = ctx.enter_context(tc.tile_pool(name="cpc", bufs=1))
        ctxn_pool = ctx.enter_context(tc.tile_pool(name="ctxn", bufs=2))
        small = ctx.enter_context(tc.tile_pool(name="small", bufs=1))
        osb_pool = ctx.enter_context(tc.tile_pool(name="osb", bufs=2))
        # PSUM: scores 2x2 banks + ctxA 1 + ctxB 1 + proj 2 = 8 banks
        st_ps = ctx.enter_context(tc.tile_pool(name="st_ps", bufs=2, space="PSUM"))
        ctx_ps = ctx.enter_context(tc.tile_pool(name="ctx_ps", bufs=1, space="PSUM"))
        proj_ps = ctx.enter_context(tc.tile_pool(name="proj_ps", bufs=2, space="PSUM"))

        # ---- weight/mask DMAs on the Activation HWDGE queue ----
        wvT, wqT, wkT = [], [], []
        for nm, lst in (("wv", wvT), ("wq", wqT), ("wk", wkT)):
            for c in range(NDC):
                t = wts.tile([128, DG], bf16, tag=f"{nm}{c}", name=f"{nm}{c}")
                nc.scalar.dma_start(out=t[:], in_=aps[nm + "T"][c * 128:(c + 1) * 128, :])
                lst.append(t)
        woT2 = []
        for p in range(NP):
            t = wts.tile([128, DM], bf16, tag=f"wo{p}", name=f"wo{p}")
            nc.scalar.dma_start(out=t[:], in_=aps["woT"][p * 128:(p + 1) * 128, :])
            woT2.append(t)
        masks = []
        for r in range(4):
            mt = wts.tile([TB, 2 * LCH], bf16, tag=f"mask{r}", name=f"mask{r}")
            nc.scalar.dma_start(out=mt[:], in_=aps["maskt"][r])
            masks.append(mt)

        # pre-zero the pt buffers (stale cols are mask-multiplied; NaN*0=NaN)
        pt_boot = []
        for _ in range(4):
            pt = pt_pool.tile([128, 2 * LCH], bf16, tag="pt")
            nc.vector.memset(pt[:], 0.0)
            pt_boot.append(pt)

        vt = [None] * NT            # [128, HG, HD+1] V tiles (ones in col HD)
        QT = [[None] * NCH for _ in range(NP)]
        KT = [[None] * NCH for _ in range(NP)]

        def attn_pair(p, i):
            """Causal attention for head-pair p over l-chunk i."""
            jmax = 4 * i + 3
            QTi = QT[p][i]
            ctxA = ctx_ps.tile([65, LCH], f32, tag="ctxA")
            ctxB = ctx_ps.tile([65, LCH], f32, tag="ctxB")
            pts = {}

            def sc_act(j):
                r = j - 4 * i
                off = 128 * r if r > 0 else 0
                sp = st_ps.tile([128, 2 * LCH], f32, tag="st")
                jn, jo = j // 4, (j % 4) * 128
                KTj = KT[p][jn]
                nc.tensor.matmul(sp[:, off:LCH], lhsT=KTj[0:64, jo:jo + 128],
                                 rhs=QTi[0:64, off:LCH], start=True, stop=True)
                nc.tensor.matmul(sp[:, LCH + off:2 * LCH], lhsT=KTj[64:128, jo:jo + 128],
                                 rhs=QTi[64:128, off:LCH], start=True, stop=True)
                pt = pt_pool.tile([128, 2 * LCH], bf16, tag="pt")
                if off == 0:
                    nc.scalar.activation(pt[:], sp[:], Exp, scale=float(SCALE))
                else:
                    nc.scalar.activation(pt[:, off:LCH], sp[:, off:LCH], Exp,
                                         scale=float(SCALE))
                    nc.scalar.activation(pt[:, LCH + off:2 * LCH],
                                         sp[:, LCH + off:2 * LCH], Exp,
                                         scale=float(SCALE))
                if r >= 0:
                    # mask cols [0:128(r+1)) per head: zeroes the causal
                    # triangle plus any stale columns; the rest is all-ones
                    w = 128 * (r + 1)
                    nc.vector.tensor_mul(pt[:, 0:w], pt[:, 0:w], masks[r][:, 0:w])
                    nc.vector.tensor_mul(pt[:, LCH:LCH + w], pt[:, LCH:LCH + w],
                                         masks[r][:, LCH:LCH + w])
                pts[j] = (pt, off)

            def pv(j):
                pt, off = pts.pop(j)
                st_f = (j == 0)
                en = (j == jmax)
                nc.tensor.matmul(ctxA[:, off:LCH], lhsT=vt[j][:, 2 * p, :],
                                 rhs=pt[:, off:LCH], start=st_f, stop=en,
                                 skip_group_check=True)
                nc.tensor.matmul(ctxB[:, off:LCH], lhsT=vt[j][:, 2 * p + 1, :],
                                 rhs=pt[:, LCH + off:2 * LCH], start=st_f, stop=en,
                                 skip_group_check=True)

            sc_act(0)
            for j in range(jmax + 1):
                if j + 1 <= jmax:
                    sc_act(j + 1)
                pv(j)
            # drain ctx+sums to SBUF, then normalize eagerly
            cA = cpc_pool.tile([65, LCH], f32, tag=f"cpcA{p}", name=f"cpcA{p}")
            cB = cpc_pool.tile([65, LCH], f32, tag=f"cpcB{p}", name=f"cpcB{p}")
            nc.vector.tensor_copy(cA[:], ctxA[:])
            nc.vector.tensor_copy(cB[:], ctxB[:])
            # 1/rowsum: gather the two sum rows to partition 0, approx-recip,
            # broadcast, then scale ctx into the combined [128, LCH] ctxn tile
            rec = small.tile([1, 2 * LCH], f32, tag="rec")
            nc.gpsimd.dma_start(out=rec[0:1, 0:LCH], in_=cA[64:65, :])
            nc.gpsimd.dma_start(out=rec[0:1, LCH:2 * LCH], in_=cB[64:65, :])
            recr = small.tile([1, 2 * LCH], f32, tag="recr")
            nc.vector.reciprocal_approx_fast(out=recr[:], in_=rec[:])
            rb = small.tile([128, 2 * LCH], f32, tag="rb")
            nc.gpsimd.partition_broadcast(rb[:], recr[0:1, :])
            tn = ctxn_pool.tile([128, LCH], bf16, tag=f"cn{p}", name=f"cn{p}")
            tB = small.tile([64, LCH], bf16, tag="cnBtmp")
            nc.gpsimd.tensor_mul(tn[0:64, :], cA[0:64, :], rb[0:64, 0:LCH])
            nc.gpsimd.tensor_mul(tB[:], cB[0:64, :], rb[0:64, LCH:2 * LCH])
            nc.gpsimd.dma_start(out=tn[64:128, :], in_=tB[:])
            return tn

        def emit_wo(m, ctxn):
            """Output projection for l-chunk m using normalized ctx tiles."""
            for ltl in range(4):
                lt = 4 * m + ltl
                osb = osb_pool.tile([128, DM], f32, tag="osb")
                for oc in range(2):
                    ps = proj_ps.tile([128, LCH], f32, tag="proj")
                    for p in range(NP):
                        nc.tensor.matmul(ps[:], lhsT=ctxn[p][:, ltl * 128:(ltl + 1) * 128],
                                         rhs=woT2[p][:, oc * LCH:(oc + 1) * LCH],
                                         start=(p == 0), stop=(p == NP - 1))
                    nc.vector.tensor_copy(osb[:, oc * LCH:(oc + 1) * LCH], ps[:])
                nc.scalar.dma_start(out=aps["y"][lt * TB:(lt + 1) * TB, :], in_=osb[:])

        ctxn_prev = None
        for n in range(NCH):
            # ---- x DMAs for chunk n (sync queue), V projection ----
            xv_n, xq_n, xk_n = [], [], []
            for c in range(NDC):
                t = xpool.tile([128, LCH], bf16, tag=f"xv{c}", name=f"xv{c}_{n}")
                nc.sync.dma_start(out=t[:], in_=aps["xvT"][c * 128:(c + 1) * 128,
                                                           n * LCH:(n + 1) * LCH])
                xv_n.append(t)
            for stl in range(4):
                st = 4 * n + stl
                ps = proj_ps.tile([128, DG], f32, tag="proj")
                for c in range(NDC):
                    nc.tensor.matmul(ps[:], lhsT=xv_n[c][:, stl * 128:(stl + 1) * 128],
                                     rhs=wvT[c][:], start=(c == 0), stop=(c == NDC - 1))
                v = vt_pool.tile([128, HG, HD + 1], bf16, tag=f"V{st}", name=f"V{st}")
                nc.vector.tensor_copy(v[:, :, 0:HD],
                                      ps[:].rearrange("a (b c) -> a b c", b=HG))
                nc.vector.memset(v[:, :, HD:HD + 1], 1.0)
                vt[st] = v
            for c in range(NDC):
                t = xpool.tile([128, LCH], bf16, tag=f"xq{c}", name=f"xq{c}_{n}")
                nc.sync.dma_start(out=t[:], in_=aps["xqT"][c * 128:(c + 1) * 128,
                                                           n * LCH:(n + 1) * LCH])
                xq_n.append(t)
            for c in range(NDC):
                t = xpool.tile([128, LCH], bf16, tag=f"xk{c}", name=f"xk{c}_{n}")
                nc.sync.dma_start(out=t[:], in_=aps["xkT"][c * 128:(c + 1) * 128,
                                                           n * LCH:(n + 1) * LCH])
                xk_n.append(t)

            # ---- Q/K projection for pair p, then its attention; Wo for the
            # previous chunk between pairs 1 and 2 ----
            ctxn_cur = [None] * NP
            for p in range(NP):
                ps = proj_ps.tile([128, LCH], f32, tag="proj")
                for c in range(NDC):
                    nc.tensor.matmul(ps[:], lhsT=wqT[c][:, p * 128:(p + 1) * 128],
                                     rhs=xq_n[c][:], start=(c == 0), stop=(c == NDC - 1))
                qt = qt_pool.tile([128, LCH], bf16, tag=f"QT{p}_{n}", name=f"QT{p}_{n}")
                nc.vector.tensor_copy(qt[:], ps[:])
                QT[p][n] = qt
                ps = proj_ps.tile([128, LCH], f32, tag="proj")
                for c in range(NDC):
                    nc.tensor.matmul(ps[:], lhsT=wkT[c][:, p * 128:(p + 1) * 128],
                                     rhs=xk_n[c][:], start=(c == 0), stop=(c == NDC - 1))
                kt = kt_pool.tile([128, LCH], bf16, tag=f"KT{p}_{n}", name=f"KT{p}_{n}")
                nc.vector.tensor_copy(kt[:], ps[:])
                KT[p][n] = kt
                ctxn_cur[p] = attn_pair(p, n)
                if p == 1 and n > 0:
                    emit_wo(n - 1, ctxn_prev)
            ctxn_prev = ctxn_cur

        emit_wo(NCH - 1, ctxn_prev)


def make_mask_tiles(cfg):
    T_, LCH_, TB_ = cfg["T"], min(512, cfg["T"]), 128
    nMask = LCH_ // TB_
    f = np.arange(2 * LCH_) % LCH_
    p = np.arange(TB_)
    tiles = []
    for r in range(nMask):
        m = (f[None, :] >= (TB_ * r + p)[:, None]).astype(np.float32)
        tiles.append(m)
    return np.stack(tiles).astype(BF16)


def build_nc(cfg):
    """Build and compile the per-core Bass program. Returns nc."""
    import concourse.bacc as bacc
    import concourse.tile as tile
    from concourse import mybir

    T_, DM_, DG_ = cfg["T"], cfg["DM"], cfg["DG"]

    nc = bacc.Bacc("TRN2", target_bir_lowering=False, debug=False)
    f32 = mybir.dt.float32
    bf16 = mybir.dt.bfloat16
    aps = {}
    for nm, shape, dt in [
        ("xqT", [DM_, T_], bf16), ("xkT", [DM_, T_], bf16), ("xvT", [DM_, T_], bf16),
        ("wqT", [DM_, DG_], bf16), ("wkT", [DM_, DG_], bf16), ("wvT", [DM_, DG_], bf16),
        ("woT", [DG_, DM_], bf16),
        ("maskt", [4, 128, 1024], bf16),
    ]:
        aps[nm] = nc.dram_tensor(nm, shape, dt, kind="ExternalInput").ap()
    aps["y"] = nc.dram_tensor("y", [T_, DM_], f32, kind="ExternalOutput").ap()

    with tile.TileContext(nc) as tc:
        emit_mha(tc, aps)
    nc.compile()
    return nc


_CACHE = {}


def _get_nc():
    if "nc" not in _CACHE:
        _CACHE["nc"] = build_nc(FULL_CFG)
    return _CACHE["nc"]


def shard_inputs(q, k, v, Wq, Wk, Wv, Wo):
    """Build the per-core input maps (8 cores = 4 batches x 2 groups)."""
    maskt = make_mask_tiles(FULL_CFG)
    xT = {}
    for b in range(B):
        xT[b] = (np.ascontiguousarray(q[b].T).astype(BF16),
                 np.ascontiguousarray(k[b].T).astype(BF16),
                 np.ascontiguousarray(v[b].T).astype(BF16))
    wT = {}
    for g in range(GROUPS):
        rows = slice(g * DG, (g + 1) * DG)
        wT[g] = (np.ascontiguousarray(Wq[rows].T).astype(BF16),
                 np.ascontiguousarray(Wk[rows].T).astype(BF16),
                 np.ascontiguousarray(Wv[rows].T).astype(BF16),
                 np.ascontiguousarray(Wo[:, rows].T).astype(BF16))
    in_maps = []
    for core in range(NCORES):
        b, g = divmod(core, GROUPS)
        xqT, xkT, xvT = xT[b]
        wqT, wkT, wvT, woT = wT[g]
        in_maps.append({
            "xqT": xqT, "xkT": xkT, "xvT": xvT,
            "wqT": wqT, "wkT": wkT, "wvT": wvT, "woT": woT,
            "maskt": maskt,
        })
    return in_maps


def kernel(q, k, v, mask, Wq, Wk, Wv, Wo):
    from concourse import bass_utils

    q = np.asarray(q, dtype=np.float32)
    k = np.asarray(k, dtype=np.float32)
    v = np.asarray(v, dtype=np.float32)
    Wq = np.asarray(Wq, dtype=np.float32)
    Wk = np.asarray(Wk, dtype=np.float32)
    Wv = np.asarray(Wv, dtype=np.float32)
    Wo = np.asarray(Wo, dtype=np.float32)

    nc = _get_nc()
    in_maps = shard_inputs(q, k, v, Wq, Wk, Wv, Wo)
    res = bass_utils.run_bass_kernel_spmd(nc, in_maps, core_ids=list(range(NCORES)))
    out = np.zeros((B, L, D), dtype=np.float32)
    for core in range(NCORES):
        b = core // GROUPS
        out[b] += res.results[core]["y"]
    return out
